# revision 4
# baseline (speedup 1.0000x reference)
"""TRN2 Bass kernel for nn_KANSpikingNeuron (2-layer MLP + spike threshold).

    h = relu(x @ W1.T + b1); y = h @ W2.T + b2; out = (y > 0).float()

Strategy
--------
- Data-parallel: batch 8192 sharded 1024 rows/core across 8 cores, weights
  replicated. No collectives.
- Numerics: the output is a threshold of y, so matmuls must be near-fp32
  or spikes flip vs the fp32 reference (bf16 flips ~1e-3 of outputs ->
  rel err 4.5e-2; measured on HW). Each matmul runs as a 3-pass bf16
  split (hi*hi + hi*lo + lo*hi accumulated in fp32 PSUM), giving ~4e-6
  matmul rel error (measured on HW) at 3 cycles/row -- vs 4 for native
  fp32 and 1 for f32r (f32r's 11-bit mantissa flips too many spikes).
- Per core: loop 2 chunks of 512 batch cols. Layer 1 computes
  h[hid, 512] = relu(W1T.T @ xT + b1) with W1 tiles stationary and xT
  moving (N=512/psum bank); epilogue splits h into bf16 hi/lo in SBUF.
  Layer 2 keeps h tiles stationary and streams W2T as the moving operand,
  accumulating y[batch 128, out 512] in 4 psum banks per out-slice; b2 is
  added exactly via one K=2 matmul with a bf16(hi)/bf16(lo) pair of rows;
  threshold is one DVE is_gt op.
- All operand layouts (transposes, hi/lo splits, 128x512 tiling) are
  prepared host-side in numpy; the device never transposes.
"""

import numpy as np
import ml_dtypes

BF16 = ml_dtypes.bfloat16

BATCH, IN_DIM, HID, OUT_DIM = 8192, 4096, 4096, 4096
NCORES = 8
CORE_B = BATCH // NCORES  # 1024
CHUNKS = 2
CB = 512  # batch columns per chunk
KT = IN_DIM // 128  # 32 contraction tiles (both layers)
MT1 = HID // 128  # 32 layer-1 output tiles
NT2 = OUT_DIM // 512  # 8 layer-2 out slices
MB2 = CB // 128  # 4 layer-2 batch tiles per chunk

LAST_RESULT = None  # BassKernelResults of the most recent run (for test.py)
LAST_NC = None
LAST_IN_MAPS = None


def _build_nc():
    import concourse.mybir as mybir
    import concourse.tile as tile
    from concourse import bacc

    dt = mybir.dt
    nc = bacc.Bacc("TRN2", target_bir_lowering=False)

    xhi_d = nc.dram_tensor("xhi", [CHUNKS, KT, 128, CB], dt.bfloat16, kind="ExternalInput")
    xlo_d = nc.dram_tensor("xlo", [CHUNKS, KT, 128, CB], dt.bfloat16, kind="ExternalInput")
    w1hi_d = nc.dram_tensor("w1hi", [MT1, KT, 128, 128], dt.bfloat16, kind="ExternalInput")
    w1lo_d = nc.dram_tensor("w1lo", [MT1, KT, 128, 128], dt.bfloat16, kind="ExternalInput")
    w2hi_d = nc.dram_tensor("w2hi", [NT2, KT, 128, 512], dt.bfloat16, kind="ExternalInput")
    w2lo_d = nc.dram_tensor("w2lo", [NT2, KT, 128, 512], dt.bfloat16, kind="ExternalInput")
    b1_d = nc.dram_tensor("b1t", [128, MT1], dt.float32, kind="ExternalInput")
    b2_d = nc.dram_tensor("b2hl", [2, OUT_DIM], dt.bfloat16, kind="ExternalInput")
    out_d = nc.dram_tensor("out", [CORE_B, OUT_DIM], dt.float32, kind="ExternalOutput")

    with tile.TileContext(nc) as tc:
        with (
            tc.tile_pool(name="xp", bufs=1) as xp,
            tc.tile_pool(name="hp", bufs=1) as hp,
            tc.tile_pool(name="wp", bufs=8) as wp,
            tc.tile_pool(name="w2p", bufs=4) as w2p,
            tc.tile_pool(name="ep", bufs=3) as ep,
            tc.tile_pool(name="op", bufs=6) as op,
            tc.tile_pool(name="cp", bufs=1) as cp,
            tc.tile_pool(name="ps", bufs=1, space="PSUM") as ps,
        ):
            b1_sb = cp.tile([128, MT1], dt.float32, name="b1_sb")
            nc.sync.dma_start(b1_sb[:], b1_d[:])
            b2_sb = cp.tile([2, OUT_DIM], dt.bfloat16, name="b2_sb")
            nc.sync.dma_start(b2_sb[:], b2_d[:])
            ones2 = cp.tile([2, 128], dt.bfloat16, name="ones2")
            nc.vector.memset(ones2[:], 1.0)

            for c in range(CHUNKS):
                # ---- layer 1: h[hid, CB] = relu(W1T.T @ xT + b1) ----
                x_t = {}
                for k in range(KT):
                    for p, src in (("hi", xhi_d), ("lo", xlo_d)):
                        t = xp.tile([128, CB], dt.bfloat16, name=f"x{p}_{k}", tag=f"x{p}_{k}")
                        nc.sync.dma_start(t[:], src[c, k])
                        x_t[p, k] = t
                h_t = {}
                for m in range(MT1):
                    p1 = ps.tile([128, CB], dt.float32, name="p1", tag="p1", bufs=2)
                    n_mm = KT * 3
                    i = 0
                    for k in range(KT):
                        whi = wp.tile([128, 128], dt.bfloat16, name="w1hi_t", tag="w1hi_t")
                        nc.sync.dma_start(whi[:], w1hi_d[m, k])
                        wlo = wp.tile([128, 128], dt.bfloat16, name="w1lo_t", tag="w1lo_t")
                        nc.sync.dma_start(wlo[:], w1lo_d[m, k])
                        for lhs, rhs in (
                            (whi, x_t["hi", k]),
                            (whi, x_t["lo", k]),
                            (wlo, x_t["hi", k]),
                        ):
                            nc.tensor.matmul(
                                p1[:], lhs[:], rhs[:],
                                start=(i == 0), stop=(i == n_mm - 1),
                            )
                            i += 1
                    h32 = ep.tile([128, CB], dt.float32, name="h32", tag="h32")
                    nc.scalar.activation(
                        h32[:], p1[:],
                        mybir.ActivationFunctionType.Relu,
                        bias=b1_sb[:, m : m + 1],
                    )
                    hhi = hp.tile([128, CB], dt.bfloat16, name=f"hhi_{m}", tag=f"hhi_{m}")
                    nc.vector.tensor_copy(hhi[:], h32[:])
                    hlo = hp.tile([128, CB], dt.bfloat16, name=f"hlo_{m}", tag=f"hlo_{m}")
                    nc.vector.tensor_sub(hlo[:], h32[:], hhi[:])
                    h_t["hi", m] = hhi
                    h_t["lo", m] = hlo

                # ---- layer 2: y[CB, OUT] = h.T @ W2T + b2; out = y > 0 ----
                for n in range(NT2):
                    p2 = [
                        ps.tile([128, 512], dt.float32, name=f"p2_{m2}", tag=f"p2_{m2}", bufs=1)
                        for m2 in range(MB2)
                    ]
                    for k in range(KT):
                        vhi = w2p.tile([128, 512], dt.bfloat16, name="w2hi_t", tag="w2hi_t")
                        nc.sync.dma_start(vhi[:], w2hi_d[n, k])
                        vlo = w2p.tile([128, 512], dt.bfloat16, name="w2lo_t", tag="w2lo_t")
                        nc.sync.dma_start(vlo[:], w2lo_d[n, k])
                        for m2 in range(MB2):
                            sl = slice(m2 * 128, (m2 + 1) * 128)
                            for j, (lhs, rhs) in enumerate(
                                (
                                    (h_t["hi", k][:, sl], vhi[:]),
                                    (h_t["hi", k][:, sl], vlo[:]),
                                    (h_t["lo", k][:, sl], vhi[:]),
                                )
                            ):
                                nc.tensor.matmul(
                                    p2[m2][:], lhs, rhs,
                                    start=(k == 0 and j == 0),
                                    stop=False,
                                )
                    for m2 in range(MB2):
                        nc.tensor.matmul(
                            p2[m2][:],
                            ones2[:],
                            b2_sb[:, n * 512 : (n + 1) * 512],
                            start=False,
                            stop=True,
                        )
                        ot = op.tile([128, 512], dt.float32, name="ot", tag="ot")
                        nc.vector.tensor_scalar(
                            ot[:], p2[m2][:], 0.0, None, mybir.AluOpType.is_gt
                        )
                        nc.sync.dma_start(
                            out_d[
                                c * CB + m2 * 128 : c * CB + (m2 + 1) * 128,
                                n * 512 : (n + 1) * 512,
                            ],
                            ot[:],
                        )
    nc.compile()
    return nc


def _split_hi_lo(a32):
    hi = a32.astype(BF16)
    lo = (a32 - hi.astype(np.float32)).astype(BF16)
    return hi, lo


def _prep_inputs(x, W1, b1, W2, b2):
    x = np.asarray(x, np.float32)
    W1 = np.asarray(W1, np.float32)
    W2 = np.asarray(W2, np.float32)
    b1 = np.asarray(b1, np.float32)
    b2 = np.asarray(b2, np.float32)

    xT = np.ascontiguousarray(x.T)  # [IN, BATCH]
    xhi, xlo = _split_hi_lo(xT)

    def arr_x(a):  # [IN, BATCH] -> [core, chunk, k, 128, CB]
        return np.ascontiguousarray(
            a.reshape(KT, 128, NCORES, CHUNKS, CB).transpose(2, 3, 0, 1, 4)
        )

    XHI, XLO = arr_x(xhi), arr_x(xlo)

    W1T = np.ascontiguousarray(W1.T)  # [IN, HID]
    w1hi, w1lo = _split_hi_lo(W1T)

    def arr_w1(a):  # [IN, HID] -> [m, k, 128, 128]
        return np.ascontiguousarray(a.reshape(KT, 128, MT1, 128).transpose(2, 0, 1, 3))

    W2T = np.ascontiguousarray(W2.T)  # [HID, OUT]
    w2hi, w2lo = _split_hi_lo(W2T)

    def arr_w2(a):  # [HID, OUT] -> [n, k, 128, 512]
        return np.ascontiguousarray(a.reshape(KT, 128, NT2, 512).transpose(2, 0, 1, 3))

    b1t = np.ascontiguousarray(b1.reshape(MT1, 128).T)  # [128, MT1]
    b2hi, b2lo = _split_hi_lo(b2)
    b2hl = np.ascontiguousarray(np.stack([b2hi, b2lo]))  # [2, OUT]

    shared = {
        "w1hi": arr_w1(w1hi),
        "w1lo": arr_w1(w1lo),
        "w2hi": arr_w2(w2hi),
        "w2lo": arr_w2(w2lo),
        "b1t": b1t,
        "b2hl": b2hl,
    }
    in_maps = []
    for core in range(NCORES):
        m = dict(shared)
        m["xhi"] = np.ascontiguousarray(XHI[core])
        m["xlo"] = np.ascontiguousarray(XLO[core])
        in_maps.append(m)
    return in_maps


def kernel(x, W1, b1, W2, b2):
    global LAST_RESULT, LAST_NC, LAST_IN_MAPS
    from concourse.bass_utils import run_bass_kernel_spmd

    nc = _build_nc()
    in_maps = _prep_inputs(x, W1, b1, W2, b2)
    LAST_NC, LAST_IN_MAPS = nc, in_maps
    res = run_bass_kernel_spmd(nc, in_maps, core_ids=list(range(NCORES)))
    LAST_RESULT = res
    out = np.concatenate([r["out"] for r in res.results], axis=0)
    return np.ascontiguousarray(out.astype(np.float32))
